# revision 2
# baseline (speedup 1.0000x reference)
"""Causal multi-head attention (B=2, S=2048, D=1024, H=16) on 8 TRN2 NeuronCores.

Sharding: batch (2-way) x head-group (4 heads, 4-way) = 8 cores. Each core
computes QKV projection for its 4 heads, causal flash-style attention, and a
partial output projection over its heads' channels; the host sums the 4
partials per batch element (tensor-parallel unshard) and adds the bias.

Per-core kernel layout (all matmuls in fp32r):
  - qT/kT per head-pair: [128, S] sbuf tiles, head A on partitions 0:64,
    head B on 64:128 -> scores^T via K=64 row-packed pair matmuls.
  - scores^T [s_k, s_q] blocks go through one wide ACT exp (scale=1/8) per
    2 k-blocks; causal masking by 0/1 mask multiply on diagonal blocks.
  - PV: M=65 matmuls with lhsT=[V_h | ones] accumulate both O^T and the
    softmax denominator (row 64) in PSUM across k-blocks.
  - normalize via reciprocal + gpsimd partition_broadcast, assemble [128, S]
    O^T pair tiles (head B moved to partitions 64:128 by SBUF->SBUF DMA).
  - output projection contracts the 256 head channels (2 pair K-tiles).
"""
import numpy as np

import concourse.bacc as bacc
import concourse.mybir as mybir
import concourse.tile as tile
from concourse.bass_utils import run_bass_kernel_spmd

F32 = mybir.dt.float32
F32R = mybir.dt.float32r
EXP = mybir.ActivationFunctionType.Exp

S = 2048          # sequence length
DK = 1024         # model dim
KT = 8            # K tiles of 128 over DK
NQB = 4           # q blocks of 512
NKB = 16          # k blocks of 128

_cache = {}


def _build():
    nc = bacc.Bacc("TRN2", target_bir_lowering=False, debug=False, num_devices=8)

    xT = nc.dram_tensor("xT", [DK, S], F32R, kind="ExternalInput").ap()
    wq = nc.dram_tensor("wq", [DK, 768], F32R, kind="ExternalInput").ap()
    wo = nc.dram_tensor("wo", [256, DK], F32R, kind="ExternalInput").ap()
    mk = nc.dram_tensor("mk", [4, 128, 512], F32, kind="ExternalInput").ap()
    ov = nc.dram_tensor("ov", [128, 16], F32R, kind="ExternalInput").ap()
    out = nc.dram_tensor("out", [S, DK], F32, kind="ExternalOutput").ap()

    xT_r = xT.rearrange("(o p) s -> p o s", p=128)      # [128, 8, S]
    wq_r = wq.rearrange("(o p) c -> p o c", p=128)      # [128, 8, 768]
    wo_r = wo.rearrange("(o p) i -> p o i", p=128)      # [128, 2, DK]
    mk_r = mk.rearrange("d p n -> p d n")               # [128, 4, 512]
    out_r = out.rearrange("(so p) i -> so p i", p=128)  # [16, 128, DK]

    with tile.TileContext(nc) as tc:
        with (
            tc.tile_pool(name="sbc", bufs=1) as sbc,
            tc.tile_pool(name="sbx", bufs=2) as sbx,
            tc.tile_pool(name="pexp", bufs=2) as pexp,
            tc.tile_pool(name="pnorm", bufs=2) as pnorm,
            tc.tile_pool(name="post", bufs=2) as post,
            tc.tile_pool(name="pqkv", bufs=2, space="PSUM") as pqkv,
            tc.tile_pool(name="psc", bufs=1, space="PSUM") as psc,
            tc.tile_pool(name="pot", bufs=2, space="PSUM") as pot,
        ):
            wqs = sbc.tile([128, KT, 768], F32R, tag="wqs")
            wos = sbc.tile([128, 2, DK], F32R, tag="wos")
            mks = sbc.tile([128, 4, 512], F32, tag="mks")
            nc.sync.dma_start(wqs[:], wq_r)
            nc.sync.dma_start(wos[:], wo_r)
            nc.sync.dma_start(mks[:], mk_r)

            qkt = [sbc.tile([128, S], F32R, tag=f"qk{m}", name=f"qk{m}")
                   for m in range(4)]
            Vp = [sbc.tile([128, 16, 130], F32R, tag=f"v{p}", name=f"v{p}")
                  for p in range(2)]
            OTp = [sbc.tile([128, S], F32R, tag=f"otp{p}", name=f"otp{p}")
                   for p in range(2)]
            for p in range(2):
                nc.sync.dma_start(Vp[p][:, :, 64:65], ov[:, :, None])
                nc.sync.dma_start(Vp[p][:, :, 129:130], ov[:, :, None])

            for qb in range(NQB):
                qsl = slice(qb * 512, (qb + 1) * 512)
                nkb = (qb + 1) * 4

                # ---- QKV projection for s-chunk qb ----
                xt = sbx.tile([128, KT, 512], F32R, tag="xt")
                nc.sync.dma_start(xt[:], xT_r[:, :, qsl])
                for m in range(4):  # q_p0, k_p0, q_p1, k_p1
                    pq = pqkv.tile([128, 512], F32, tag="ps")
                    for kt in range(KT):
                        nc.tensor.matmul(
                            pq[:],
                            lhsT=wqs[:, kt, m * 128:(m + 1) * 128],
                            rhs=xt[:, kt, :],
                            start=(kt == 0), stop=(kt == KT - 1),
                        )
                    nc.vector.tensor_copy(qkt[m][:, qsl], pq[:])
                for sc in range(4):  # v, s-major: [s 128, 256]
                    pv = pqkv.tile([128, 512], F32, tag="ps")
                    for kt in range(KT):
                        nc.tensor.matmul(
                            pv[:, 0:256],
                            lhsT=xt[:, kt, sc * 128:(sc + 1) * 128],
                            rhs=wqs[:, kt, 512:768],
                            start=(kt == 0), stop=(kt == KT - 1),
                        )
                    so = qb * 4 + sc
                    nc.vector.tensor_copy(Vp[0][:, so, 0:64], pv[:, 0:64])
                    nc.vector.tensor_copy(Vp[0][:, so, 65:129], pv[:, 64:128])
                    nc.vector.tensor_copy(Vp[1][:, so, 0:64], pv[:, 128:192])
                    nc.vector.tensor_copy(Vp[1][:, so, 65:129], pv[:, 192:256])

                # ---- attention for q-block qb ----
                for p in range(2):
                    qT, kT_, V = qkt[2 * p], qkt[2 * p + 1], Vp[p]
                    otA = pot.tile([65, 512], F32, tag="ot")
                    otB = pot.tile([65, 512], F32, tag="ot")
                    for kbg in range(nkb // 2):
                        pse = psc.tile([128, 2048], F32, tag="sc")
                        for j in range(2):
                            kb = kbg * 2 + j
                            ksl = slice(kb * 128, (kb + 1) * 128)
                            nc.tensor.matmul(
                                pse[:, j * 1024:j * 1024 + 512],
                                lhsT=kT_[0:64, ksl], rhs=qT[0:64, qsl],
                                start=True, stop=True,
                            )
                            nc.tensor.matmul(
                                pse[:, j * 1024 + 512:j * 1024 + 1024],
                                lhsT=kT_[64:128, ksl], rhs=qT[64:128, qsl],
                                start=True, stop=True,
                            )
                        texp = pexp.tile([128, 2048], F32R, tag="exp")
                        nc.scalar.activation(texp[:], pse[:], EXP, scale=0.125)
                        for j in range(2):
                            kb = kbg * 2 + j
                            i = kb - qb * 4
                            slA = slice(j * 1024, j * 1024 + 512)
                            slB = slice(j * 1024 + 512, j * 1024 + 1024)
                            if i >= 0:  # diagonal block: causal mask
                                nc.vector.tensor_mul(
                                    texp[:, slA], texp[:, slA], mks[:, i, :])
                                nc.vector.tensor_mul(
                                    texp[:, slB], texp[:, slB], mks[:, i, :])
                            nc.tensor.matmul(
                                otA[:], lhsT=V[:, kb, 0:65], rhs=texp[:, slA],
                                start=(kb == 0), stop=(kb == nkb - 1),
                            )
                            nc.tensor.matmul(
                                otB[:], lhsT=V[:, kb, 65:130], rhs=texp[:, slB],
                                start=(kb == 0), stop=(kb == nkb - 1),
                            )
                    # normalize: rows 0:64 = O^T, row 64 = sum(exp)
                    for h, ot in ((0, otA), (1, otB)):
                        l64 = pnorm.tile([65, 512], F32, tag="l64")
                        nc.vector.tensor_copy(l64[64:65, :], ot[64:65, :])
                        l0 = pnorm.tile([1, 512], F32, tag="l0")
                        nc.sync.dma_start(l0[:], l64[64:65, :])
                        rec = pnorm.tile([1, 512], F32, tag="rec")
                        nc.vector.reciprocal(rec[:], l0[:])
                        bch = pnorm.tile([64, 512], F32, tag="bch")
                        nc.gpsimd.partition_broadcast(bch[:], rec[:])
                        if h == 0:
                            nc.vector.tensor_mul(
                                OTp[p][0:64, qsl], ot[0:64, :], bch[:])
                        else:
                            tmpB = pnorm.tile([64, 512], F32R, tag="tmpB")
                            nc.vector.tensor_mul(tmpB[:], ot[0:64, :], bch[:])
                            nc.sync.dma_start(OTp[p][64:128, qsl], tmpB[:])

                # ---- output projection for s-chunk qb ----
                for sc in range(4):
                    so = qb * 4 + sc
                    ssl = slice(qb * 512 + sc * 128, qb * 512 + (sc + 1) * 128)
                    for nb in range(2):
                        po = pqkv.tile([128, 512], F32, tag="ps")
                        for p in range(2):
                            nc.tensor.matmul(
                                po[:],
                                lhsT=OTp[p][:, ssl],
                                rhs=wos[:, p, nb * 512:(nb + 1) * 512],
                                start=(p == 0), stop=(p == 1),
                            )
                        ost = post.tile([128, 512], F32, tag="ost")
                        nc.vector.tensor_copy(ost[:], po[:])
                        nc.sync.dma_start(
                            out_r[so, :, nb * 512:(nb + 1) * 512], ost[:])

    nc.compile()
    return nc


def _masks():
    k = np.arange(128)[:, None]
    q = np.arange(512)[None, :]
    return np.stack(
        [(q >= k + 128 * i).astype(np.float32) for i in range(4)]
    )  # [4, 128, 512]


def _in_maps(x, w_qkv, w_out):
    mk = _masks()
    ov = np.ones((128, 16), dtype=np.float32)
    maps = []
    for core in range(8):
        b, g = core // 4, core % 4
        rows = []
        for p in range(2):
            ha, hb = 4 * g + 2 * p, 4 * g + 2 * p + 1
            rows.append(np.r_[192 * ha:192 * ha + 64, 192 * hb:192 * hb + 64])
            rows.append(np.r_[192 * ha + 64:192 * ha + 128,
                              192 * hb + 64:192 * hb + 128])
        for p in range(2):
            ha, hb = 4 * g + 2 * p, 4 * g + 2 * p + 1
            rows.append(np.r_[192 * ha + 128:192 * ha + 192,
                              192 * hb + 128:192 * hb + 192])
        row_order = np.concatenate(rows)
        maps.append({
            "xT": np.ascontiguousarray(x[b].T),
            "wq": np.ascontiguousarray(w_qkv[row_order, :].T),
            "wo": np.ascontiguousarray(w_out[:, 256 * g:256 * (g + 1)].T),
            "mk": mk,
            "ov": ov,
        })
    return maps


last_results = None


def kernel(x, w_qkv, w_out, b_out):
    global last_results
    x = np.ascontiguousarray(np.asarray(x, dtype=np.float32))
    w_qkv = np.ascontiguousarray(np.asarray(w_qkv, dtype=np.float32))
    w_out = np.ascontiguousarray(np.asarray(w_out, dtype=np.float32))
    b_out = np.asarray(b_out, dtype=np.float32)

    if "nc" not in _cache:
        _cache["nc"] = _build()
    nc = _cache["nc"]

    last_results = run_bass_kernel_spmd(
        nc, _in_maps(x, w_qkv, w_out), core_ids=list(range(8))
    )
    res = last_results.results
    B = x.shape[0]
    out = np.zeros((B, S, DK), dtype=np.float32)
    for core in range(8):
        out[core // 4] += res[core]["out"]
    out += b_out[None, None, :]
    return out


# revision 23
# speedup vs baseline: 2.0043x; 2.0043x over previous
"""Causal multi-head attention (B=2, S=2048, D=1024, H=16) on 8 TRN2 NeuronCores.

Sharding: batch (2-way) x head-group (4 heads, 4-way) = 8 cores. Each core
computes the QKV projection for its 4 heads, causal flash-style attention, and
a partial output projection over its heads' channels; the host sums the 4
partials per batch element (tensor-parallel unshard) and adds the bias.

Per-core kernel (all matmuls fp32r):
  - qT/kT per head-pair: [128, S] tiles, head A on partitions 0:64, head B on
    64:128 -> scores^T via K=64 row-packed pair matmuls.
  - per k-block: one [128, 1024] PSUM (head A | head B), one wide ACT exp
    (scale=1/8); causal masking = gpsimd zero-fill + one [128,128] triangle
    multiply on diagonal blocks only.
  - PV: M=65 matmuls with lhsT=[V_h | ones] accumulate O^T and the softmax
    denominator (row 64) in PSUM across k-blocks.
  - softmax denominators are lane-spread ([1,512] -> [128,4] via DMA) so the
    DVE reciprocal runs 8-elements-per-lane instead of 512.
  - output projection contracts the 256 head channels (2 pair K-tiles).
"""
import numpy as np

import concourse.bacc as bacc
import concourse.mybir as mybir
import concourse.tile as tile
from concourse.bass_utils import run_bass_kernel_spmd

F32 = mybir.dt.float32
F32R = mybir.dt.float32r
EXP = mybir.ActivationFunctionType.Exp

S = 2048          # sequence length
DK = 1024         # model dim
KT = 8            # K tiles of 128 over DK
NQB = 4           # q blocks of 512

_cache = {}


def _build():
    nc = bacc.Bacc("TRN2", target_bir_lowering=False, debug=False, num_devices=8)

    xT = nc.dram_tensor("xT", [DK, S], F32R, kind="ExternalInput").ap()
    wq = nc.dram_tensor("wq", [DK, 768], F32R, kind="ExternalInput").ap()
    wo = nc.dram_tensor("wo", [256, DK], F32R, kind="ExternalInput").ap()
    mk = nc.dram_tensor("mk", [128, 128], F32, kind="ExternalInput").ap()
    ov = nc.dram_tensor("ov", [128, 64], F32R, kind="ExternalInput").ap()
    out = nc.dram_tensor("out", [S, DK], F32, kind="ExternalOutput").ap()

    xT_r = xT.rearrange("(o p) s -> p o s", p=128)      # [128, 8, S]
    wq_r = wq.rearrange("(o p) c -> p o c", p=128)      # [128, 8, 768]
    wo_r = wo.rearrange("(o p) i -> p o i", p=128)      # [128, 2, DK]
    out_r = out.rearrange("(so p) i -> so p i", p=128)  # [16, 128, DK]

    with tile.TileContext(nc) as tc:
        with (
            tc.tile_pool(name="sbc", bufs=1) as sbc,
            tc.tile_pool(name="sbx", bufs=2) as sbx,
            tc.tile_pool(name="pexp", bufs=5) as pexp,
            tc.tile_pool(name="pnorm", bufs=2) as pnorm,
            tc.tile_pool(name="post", bufs=2) as post,
            tc.tile_pool(name="pqkv", bufs=2, space="PSUM") as pqkv,
            tc.tile_pool(name="psc", bufs=2, space="PSUM") as psc,
            tc.tile_pool(name="pot", bufs=2, space="PSUM") as pot,
        ):
            wtile = sbc.tile([128, 512], F32R, tag="wtile")
            nc.vector.memset(wtile[:].bitcast(F32), 0.5)
            for _ in range(14):
                wps = pqkv.tile([128, 512], F32, tag="ps", name="wps")
                nc.tensor.matmul(wps[:], lhsT=wtile[:, 0:128], rhs=wtile[:],
                                 start=True, stop=True)
            wqs = [sbc.tile([128, 768], F32R, tag=f"wqs{kt}", name=f"wqs{kt}")
                   for kt in range(KT)]
            wos = sbc.tile([128, 2, DK], F32R, tag="wos")
            tri = sbc.tile([128, 128], F32, tag="tri")

            qkt = [sbc.tile([128, S], F32R, tag=f"qk{m}", name=f"qk{m}")
                   for m in range(4)]
            Vp = [sbc.tile([128, 16, 130], F32R, tag=f"v{p}", name=f"v{p}")
                  for p in range(2)]
            OTp = [sbc.tile([128, S], F32R, tag=f"otp{p}", name=f"otp{p}")
                   for p in range(2)]
            ovs = sbc.tile([128, 64], F32R, tag="ovs")
            def emit_consts():
                nc.sync.dma_start(wos[:], wo_r)
                nc.sync.dma_start(tri[:], mk)
                nc.sync.dma_start(ovs[:], ov)
                for p in range(2):
                    nc.sync.dma_start(Vp[p][:, :, 64:65], ov[:, 0:16, None])
                    nc.sync.dma_start(Vp[p][:, :, 129:130], ov[:, 0:16, None])

            def emit_qkv(qb):
                qsl = slice(qb * 512, (qb + 1) * 512)
                if qb == 0:
                    # fine-grained per-kt tiles: first matmul starts after
                    # just one wq k-slice + one x k-slice has landed
                    xts = [sbc.tile([128, 512], F32R, tag=f"x0k{kt}",
                                    name=f"x0k{kt}") for kt in range(KT)]
                    for kt in range(KT):
                        nc.sync.dma_start(wqs[kt][:], wq_r[:, kt, :])
                        nc.sync.dma_start(xts[kt][:], xT_r[:, kt, qsl])
                    xsl = lambda kt: xts[kt][:]
                    xv = lambda kt, sc: xts[kt][:, sc * 128:(sc + 1) * 128]
                else:
                    xt = sbx.tile([128, KT, 512], F32R, tag="xt", name="xt")
                    nc.sync.dma_start(xt[:], xT_r[:, :, qsl])
                    xsl = lambda kt: xt[:, kt, :]
                    xv = lambda kt, sc: xt[:, kt, sc * 128:(sc + 1) * 128]
                for m in range(4):  # q_p0, k_p0, q_p1, k_p1
                    pq = pqkv.tile([128, 512], F32, tag="ps", name="pq")
                    for kt in range(KT):
                        nc.tensor.matmul(
                            pq[:],
                            lhsT=wqs[kt][:, m * 128:(m + 1) * 128],
                            rhs=xsl(kt),
                            start=(kt == 0), stop=(kt == KT - 1),
                        )
                    nc.vector.tensor_copy(qkt[m][:, qsl], pq[:])
                for sc in range(4):  # v, s-major: [s 128, 256]
                    pv = pqkv.tile([128, 512], F32, tag="ps", name="pv")
                    for kt in range(KT):
                        nc.tensor.matmul(
                            pv[:, 0:256],
                            lhsT=xv(kt, sc),
                            rhs=wqs[kt][:, 512:768],
                            start=(kt == 0), stop=(kt == KT - 1),
                        )
                    so = qb * 4 + sc
                    nc.vector.tensor_copy(Vp[0][:, so, 0:64], pv[:, 0:64])
                    nc.vector.tensor_copy(Vp[0][:, so, 65:129], pv[:, 64:128])
                    nc.vector.tensor_copy(Vp[1][:, so, 0:64], pv[:, 128:192])
                    nc.vector.tensor_copy(Vp[1][:, so, 65:129], pv[:, 192:256])

            def emit_attn_range(p, q0, qw):
                qT, kT_, V = qkt[2 * p], qkt[2 * p + 1], Vp[p]
                nkb = (q0 + qw) // 128
                otA = pot.tile([65, 512], F32, tag="ot", name="otA")[:, 0:qw]
                otB = pot.tile([65, 512], F32, tag="ot", name="otB")[:, 0:qw]
                for kb in range(nkb):
                    ksl = slice(kb * 128, (kb + 1) * 128)
                    lo = kb * 128 - q0
                    ioff = max(lo, 0)    # first causally-valid q col
                    w = qw - ioff
                    qs = q0 + ioff
                    pse = psc.tile([128, 1024], F32, tag="sc", name="pse")
                    nc.tensor.matmul(
                        pse[:, 0:w],
                        lhsT=kT_[0:64, ksl], rhs=qT[0:64, qs:qs + w],
                        start=True, stop=True,
                    )
                    nc.tensor.matmul(
                        pse[:, 512:512 + w],
                        lhsT=kT_[64:128, ksl], rhs=qT[64:128, qs:qs + w],
                        start=True, stop=True,
                    )
                    if lo >= 0:  # diagonal: add -240 above the diagonal
                        nc.vector.tensor_add(pse[:, 0:128], pse[:, 0:128],
                                             tri[:])
                        nc.vector.tensor_add(pse[:, 512:640],
                                             pse[:, 512:640], tri[:])
                    texp = pexp.tile([128, 1024], F32R, tag="exp",
                                     name="texp")
                    if w < 320:
                        nc.scalar.activation(texp[:, 0:w], pse[:, 0:w],
                                             EXP, scale=0.125)
                        nc.scalar.activation(texp[:, 512:512 + w],
                                             pse[:, 512:512 + w],
                                             EXP, scale=0.125)
                    else:
                        nc.scalar.activation(texp[:], pse[:], EXP,
                                             scale=0.125)
                    nc.tensor.matmul(
                        otA[:, ioff:qw], lhsT=V[:, kb, 0:65],
                        rhs=texp[:, 0:w],
                        start=(kb == 0), stop=(kb == nkb - 1),
                        skip_group_check=True,
                    )
                    nc.tensor.matmul(
                        otB[:, ioff:qw], lhsT=V[:, kb, 65:130],
                        rhs=texp[:, 512:512 + w],
                        start=(kb == 0), stop=(kb == nkb - 1),
                        skip_group_check=True,
                    )
                # free the OT psum banks asap: copy [65,qw] to SBUF, then
                # normalize from the copies (rows 0:64 = O^T, row 64 = denom).
                nsp = qw // 128
                sA = pnorm.tile([65, 512], F32, tag="sA", name="sA")[:, 0:qw]
                sB = pnorm.tile([65, 512], F32, tag="sB", name="sB")[:, 0:qw]
                nc.vector.tensor_copy(sA[:], otA[:])
                nc.vector.tensor_copy(sB[:], otB[:])
                lt = pnorm.tile([128, 8], F32, tag="lt", name="lt")
                nc.sync.dma_start(lt[:, 0:nsp], sA[64:65, :])
                nc.sync.dma_start(lt[:, 4:4 + nsp], sB[64:65, :])
                lti = pnorm.tile([128, 8], F32, tag="lti", name="lti")
                nc.vector.reciprocal(lti[:], lt[:])
                linvA = pnorm.tile([1, 512], F32, tag="linvA",
                                   name="lnA")[:, 0:qw]
                linvB = pnorm.tile([1, 512], F32, tag="linvB",
                                   name="lnB")[:, 0:qw]
                nc.sync.dma_start(linvA[:], lti[:, 0:nsp])
                nc.sync.dma_start(linvB[:], lti[:, 4:4 + nsp])
                bchA = pnorm.tile([64, 512], F32, tag="bchA",
                                  name="bcA")[:, 0:qw]
                bchB = pnorm.tile([64, 512], F32, tag="bchB",
                                  name="bcB")[:, 0:qw]
                nc.gpsimd.partition_broadcast(bchA[:], linvA[:])
                nc.gpsimd.partition_broadcast(bchB[:], linvB[:])
                nc.vector.tensor_mul(OTp[p][0:64, q0:q0 + qw], sA[0:64, :],
                                     bchA[:])
                tmpB = pnorm.tile([64, 512], F32R, tag="tmpB",
                                  name="tmpB")[:, 0:qw]
                nc.vector.tensor_mul(tmpB[:], sB[0:64, :], bchB[:])
                nc.sync.dma_start(OTp[p][64:128, q0:q0 + qw], tmpB[:])

            def emit_attn(qb):
                for p in range(2):
                    emit_attn_range(p, qb * 512, 512)

            def emit_outproj(qb):
                for sc in range(4):
                    so = qb * 4 + sc
                    ssl = slice(qb * 512 + sc * 128, qb * 512 + (sc + 1) * 128)
                    for nb in range(2):
                        po = pqkv.tile([128, 512], F32, tag="ps", name="po")
                        for p in range(2):
                            nc.tensor.matmul(
                                po[:],
                                lhsT=OTp[p][:, ssl],
                                rhs=wos[:, p, nb * 512:(nb + 1) * 512],
                                start=(p == 0), stop=(p == 1),
                            )
                        ost = post.tile([128, 512], F32, tag="ost", name="ost")
                        nc.vector.tensor_copy(ost[:], po[:])
                        nc.sync.dma_start(
                            out_r[so, :, nb * 512:(nb + 1) * 512], ost[:])

            with nc.named_scope("qkv0"):
                emit_qkv(0)
            emit_consts()
            for qb in range(NQB):
                with nc.named_scope(f"attn{qb}"):
                    emit_attn(qb)
                if qb + 1 < NQB:
                    with nc.named_scope(f"qkv{qb + 1}"):
                        emit_qkv(qb + 1)
                if qb >= 1:
                    with nc.named_scope(f"out{qb - 1}"):
                        emit_outproj(qb - 1)
            with nc.named_scope("out3"):
                emit_outproj(3)

    nc.compile()
    return nc


def _in_maps(x, w_qkv, w_out):
    k = np.arange(128)[:, None]
    q = np.arange(128)[None, :]
    tri = np.where(q >= k, 0.0, -240.0).astype(np.float32)  # additive mask
    ov = np.ones((128, 64), dtype=np.float32)
    maps = []
    for core in range(8):
        b, g = core // 4, core % 4
        rows = []
        for p in range(2):
            ha, hb = 4 * g + 2 * p, 4 * g + 2 * p + 1
            rows.append(np.r_[192 * ha:192 * ha + 64, 192 * hb:192 * hb + 64])
            rows.append(np.r_[192 * ha + 64:192 * ha + 128,
                              192 * hb + 64:192 * hb + 128])
        for p in range(2):
            ha, hb = 4 * g + 2 * p, 4 * g + 2 * p + 1
            rows.append(np.r_[192 * ha + 128:192 * ha + 192,
                              192 * hb + 128:192 * hb + 192])
        row_order = np.concatenate(rows)
        maps.append({
            "xT": np.ascontiguousarray(x[b].T),
            "wq": np.ascontiguousarray(w_qkv[row_order, :].T),
            "wo": np.ascontiguousarray(w_out[:, 256 * g:256 * (g + 1)].T),
            "mk": tri,
            "ov": ov,
        })
    return maps


last_results = None


def kernel(x, w_qkv, w_out, b_out):
    global last_results
    x = np.ascontiguousarray(np.asarray(x, dtype=np.float32))
    w_qkv = np.ascontiguousarray(np.asarray(w_qkv, dtype=np.float32))
    w_out = np.ascontiguousarray(np.asarray(w_out, dtype=np.float32))
    b_out = np.asarray(b_out, dtype=np.float32)

    if "nc" not in _cache:
        _cache["nc"] = _build()
    nc = _cache["nc"]

    last_results = run_bass_kernel_spmd(
        nc, _in_maps(x, w_qkv, w_out), core_ids=list(range(8))
    )
    res = last_results.results
    B = x.shape[0]
    out = np.zeros((B, S, DK), dtype=np.float32)
    for core in range(8):
        out[core // 4] += res[core]["out"]
    out += b_out[None, None, :]
    return out
